# revision 9
# baseline (speedup 1.0000x reference)
"""Trainium2 Bass kernel for nn_DPLayer: grid-DAG shortest path per sample.

Math: the reference DP with edge weights (img[u]+img[v])/2 is, after the
node-potential substitution z[i,j] = v[i,j] + img[i,j]/2, exactly the
classic node-weighted minimal-path-sum recurrence

    z[i,j] = img[i,j] + min(z[i,j-1], z[i-1,j], z[i-1,j-1])
    answer = z[H-1,W-1] - img[H-1,W-1]/2
    row 0:  z[0,j] = prefix_sum(img[0,:j]) - img[0,0]/2  (handled by the
            same scan with a special data0 row)

which maps onto the DVE TensorTensorScan instruction
    state = min(data0[t], state) + data1[t]
with data0 = m (the min of the previous z row and its 1-shifted self) and
data1 = the current image row: two vector ops per grid row.

Layout per core: 256 samples -> 128 partitions x 2 samples along the free
axis. The two samples of a partition form two INDEPENDENT serial chains
(A = free cols 0..63, B = 64..127) whose ops are interleaved in program
order; while chain A's op drains its pipeline + propagates its semaphore,
chain B's op executes, so the DVE stays busy.
"""

import sys

import numpy as np

sys.path.insert(0, "/opt/trn_rl_repo")

import concourse.bacc as bacc
import concourse.mybir as mybir
import concourse.tile as tile
from concourse.bass_utils import run_bass_kernel_spmd

P = 128          # SBUF partitions
Q = 2            # samples per partition == independent chains
H = 64
W = 64
NB_CORE = P * Q  # samples per core
N_CORES = 8
BIG = 1.0e9
F32 = mybir.dt.float32

_CACHE = {}


def _build():
    nc = bacc.Bacc("TRN2", debug=False, target_bir_lowering=False,
                   num_devices=N_CORES)
    img_d = nc.dram_tensor("images", [NB_CORE, H, W], F32,
                           kind="ExternalInput").ap()
    out_d = nc.dram_tensor("out", [P, Q], F32, kind="ExternalOutput").ap()

    with tile.TileContext(nc) as tc:
        with tc.tile_pool(name="img", bufs=1) as imgp, \
             tc.tile_pool(name="state", bufs=1) as statep, \
             tc.tile_pool(name="work", bufs=4) as workp:
            # imgT[p, r, q*W + c] = images[q*P + p, r, c] -- per (row, chain)
            # a contiguous [P, W] view (the scan requires 2D operands).
            imgT = imgp.tile([P, H, Q * W], F32)
            # zbuf[q]: per-chain z row with a BIG guard in col 0
            zbufs = [statep.tile([P, W + 1], F32, tag=f"z{q}", name=f"zbuf{q}")
                     for q in range(Q)]
            c0s = [statep.tile([P, W], F32, tag=f"c0{q}", name=f"c0_{q}")
                   for q in range(Q)]
            o = statep.tile([P, Q], F32)
            t = statep.tile([P, Q], F32)

            def img_row(r, q):
                return imgT[:, r, q * W:(q + 1) * W]

            # Stream the image in: 8-row chunks, chain A on the SP HWDGE
            # ring, chain B on the Act ring.
            RCHUNK = 8
            for r0 in range(0, H, RCHUNK):
                nc.sync.dma_start(
                    out=imgT[:, r0:r0 + RCHUNK, 0:W],
                    in_=img_d[0:P, r0:r0 + RCHUNK, :])
                nc.scalar.dma_start(
                    out=imgT[:, r0:r0 + RCHUNK, W:2 * W],
                    in_=img_d[P:2 * P, r0:r0 + RCHUNK, :])

            # Prologue per chain: guard col + row-0 scan data0 =
            # [-img00/2, BIG, BIG, ...] -> z0 = cumsum(img row 0) - img00/2.
            for q in range(Q):
                nc.vector.memset(zbufs[q][:, 0:1], BIG)
                nc.vector.memset(c0s[q][:], BIG)
                nc.vector.tensor_scalar_mul(c0s[q][:, 0:1],
                                            imgT[:, 0, q * W:q * W + 1], -0.5)
            for q in range(Q):
                nc.vector.tensor_tensor_scan(
                    out=zbufs[q][:, 1:], data0=c0s[q][:], data1=img_row(0, q),
                    initial=BIG, op0=mybir.AluOpType.min,
                    op1=mybir.AluOpType.add)

            for r in range(1, H):
                ms = []
                for q in range(Q):
                    m = workp.tile([P, W], F32, tag=f"m{q}", name=f"m{q}_{r}")
                    nc.vector.tensor_tensor(out=m[:], in0=zbufs[q][:, 1:],
                                            in1=zbufs[q][:, 0:W],
                                            op=mybir.AluOpType.min)
                    ms.append(m)
                for q in range(Q):
                    nc.vector.tensor_tensor_scan(
                        out=zbufs[q][:, 1:], data0=ms[q][:],
                        data1=img_row(r, q),
                        initial=BIG, op0=mybir.AluOpType.min,
                        op1=mybir.AluOpType.add)

            # out[:, q] = z_q[last col] - img[H-1,W-1]/2
            nc.vector.tensor_scalar_mul(t[:], imgT[:, H - 1, W - 1::W], -0.5)
            for q in range(Q):
                nc.vector.tensor_tensor(out=o[:, q:q + 1],
                                        in0=zbufs[q][:, W:W + 1],
                                        in1=t[:, q:q + 1],
                                        op=mybir.AluOpType.add)
            nc.sync.dma_start(out=out_d, in_=o[:])
    nc.compile()
    return nc


def get_nc():
    if "nc" not in _CACHE:
        _CACHE["nc"] = _build()
    return _CACHE["nc"]


def kernel(images: np.ndarray, **run_kwargs) -> np.ndarray:
    B = images.shape[0]
    assert images.shape == (B, H, W) and B == N_CORES * NB_CORE
    images = np.ascontiguousarray(images, dtype=np.float32)
    nc = get_nc()
    in_maps = [{"images": images[c * NB_CORE:(c + 1) * NB_CORE]}
               for c in range(N_CORES)]
    res = run_bass_kernel_spmd(nc, in_maps, core_ids=list(range(N_CORES)),
                               **run_kwargs)
    out = np.empty((B,), dtype=np.float32)
    for c in range(N_CORES):
        # res tile is [P, Q]; sample index within the core is q*P + p
        out[c * NB_CORE:(c + 1) * NB_CORE] = res.results[c]["out"].T.reshape(-1)
    if run_kwargs:
        return out, res
    return out


# revision 10
# speedup vs baseline: 1.1589x; 1.1589x over previous
"""Meet-in-the-middle variant: forward DP rows 0..31 + backward DP rows
63..32, combined at the row-31/32 seam. The backward chain is a forward
DP on reversed-row, column-reversed views of the same image tile (2D
negative-stride APs), so both chains run full-width (2-sample-packed,
BIAS-guarded) and the per-op SBUF-access cost is amortized over 128
elements instead of 64. In the backward chain's packed layout, slot 0
holds sample 1 (column-flipped) and slot 1 holds sample 0, so each
sample's seam sum zf+zb carries exactly one +BIAS from each side.
"""

import sys

import numpy as np

sys.path.insert(0, "/opt/trn_rl_repo")

import concourse.bacc as bacc
import concourse.mybir as mybir
import concourse.tile as tile
from concourse.bass_utils import run_bass_kernel_spmd

P = 128
Q = 2
H = 64
W = 64
HH = H // 2      # rows per direction
NB_CORE = P * Q
N_CORES = 8
BIG = 1.0e9
BIAS = 512.0     # slot-0 offset so scan carry can't leak across samples
F32 = mybir.dt.float32
MIN = mybir.AluOpType.min
ADD = mybir.AluOpType.add

_CACHE = {}


def _build():
    nc = bacc.Bacc("TRN2", debug=False, target_bir_lowering=False,
                   num_devices=N_CORES)
    img_d = nc.dram_tensor("images", [NB_CORE, H, W], F32,
                           kind="ExternalInput").ap()
    out_d = nc.dram_tensor("out", [P, Q], F32, kind="ExternalOutput").ap()

    with tile.TileContext(nc) as tc:
        with tc.tile_pool(name="img", bufs=1) as imgp, \
             tc.tile_pool(name="state", bufs=1) as statep, \
             tc.tile_pool(name="work", bufs=4) as workp:
            imgT = imgp.tile([P, H, Q * W], F32)
            zbs = {d: statep.tile([P, Q * W + 1], F32, name=f"zb_{d}")
                   for d in "FB"}
            c0s = {d: statep.tile([P, Q * W], F32, name=f"c0_{d}")
                   for d in "FB"}
            t1 = statep.tile([P, Q * W], F32)
            t2 = statep.tile([P, Q * W], F32)
            red = statep.tile([P, Q], F32)

            def img_row(d, r):
                if d == "F":
                    return imgT[:, r, :]
                return imgT[:, H - 1 - r, ::-1]

            # DMA: alternate chunks from both ends so each chain's next
            # rows arrive just ahead of consumption.
            RC = 4
            for r0 in range(0, HH, RC):
                for a, b in ((r0, r0 + RC), (H - r0 - RC, H - r0)):
                    nc.sync.dma_start(
                        out=imgT[:, a:b, 0:W], in_=img_d[0:P, a:b, :])
                    nc.scalar.dma_start(
                        out=imgT[:, a:b, W:2 * W], in_=img_d[P:2 * P, a:b, :])

            # c0 row-0 scan seed: [-start_node/2 (+BIAS in slot 0), BIG...]
            for d in "FB":
                nc.vector.memset(zbs[d][:, 0:1], BIG)
                nc.vector.memset(c0s[d][:], BIG)
                if d == "F":
                    starts = imgT[:, 0, 0:Q * W:W]          # img[q, 0, 0]
                else:
                    starts = imgT[:, H - 1, Q * W - 1::-W]  # img[1-q, 63, 63]
                nc.vector.tensor_scalar_mul(c0s[d][:, 0:Q * W:W], starts,
                                            -0.5)
                nc.vector.tensor_scalar_add(c0s[d][:, 0:1], c0s[d][:, 0:1],
                                            BIAS)
            for d in "FB":
                nc.vector.tensor_tensor_scan(
                    out=zbs[d][:, 1:], data0=c0s[d][:], data1=img_row(d, 0),
                    initial=BIG, op0=MIN, op1=ADD)

            for r in range(1, HH):
                ms = {}
                for d in "FB":
                    m = workp.tile([P, Q * W], F32, tag=f"m{d}",
                                   name=f"m{d}_{r}")
                    nc.vector.tensor_tensor(out=m[:], in0=zbs[d][:, 1:],
                                            in1=zbs[d][:, 0:Q * W], op=MIN)
                    ms[d] = m
                for d in "FB":
                    nc.vector.tensor_tensor_scan(
                        out=zbs[d][:, 1:], data0=ms[d][:],
                        data1=img_row(d, r), initial=BIG, op0=MIN, op1=ADD)

            # Seam: ans_q = min_j min(zf_q[j]+zb_q[j], zf_q[j]+zb_q[j+1]).
            # zb_q[j] lives at B-slot (1-q), position 63-j -> the doubly
            # reversed view aligns it with zf.
            zf3 = zbs["F"][:, 1:].rearrange("p (q c) -> p q c", q=Q)
            zb3 = zbs["B"][:, 1:].rearrange("p (q c) -> p q c", q=Q)
            zb_rev = zb3[:, ::-1, ::-1]
            t13 = t1[:].rearrange("p (q c) -> p q c", q=Q)
            t23 = t2[:].rearrange("p (q c) -> p q c", q=Q)
            nc.vector.memset(t2[:], BIG)
            nc.vector.tensor_tensor(out=t13[:], in0=zf3, in1=zb_rev, op=ADD)
            nc.vector.tensor_tensor(out=t23[:, :, 0:W - 1],
                                    in0=zf3[:, :, 0:W - 1],
                                    in1=zb_rev[:, :, 1:W], op=ADD)
            nc.vector.tensor_tensor(out=t1[:], in0=t1[:], in1=t2[:], op=MIN)
            nc.vector.tensor_reduce(out=red[:], in_=t13,
                                    axis=mybir.AxisListType.X, op=MIN)
            # each sample's seam sum carries exactly one +BIAS (from F for
            # sample 0, from the B chain's slot 0 for sample 1)
            nc.vector.tensor_scalar_add(red[:], red[:], -BIAS)
            nc.sync.dma_start(out=out_d, in_=red[:])
    nc.compile()
    return nc


def get_nc():
    if "nc" not in _CACHE:
        _CACHE["nc"] = _build()
    return _CACHE["nc"]


def kernel(images: np.ndarray, **run_kwargs) -> np.ndarray:
    B = images.shape[0]
    assert images.shape == (B, H, W) and B == N_CORES * NB_CORE
    images = np.ascontiguousarray(images, dtype=np.float32)
    nc = get_nc()
    in_maps = [{"images": images[c * NB_CORE:(c + 1) * NB_CORE]}
               for c in range(N_CORES)]
    res = run_bass_kernel_spmd(nc, in_maps, core_ids=list(range(N_CORES)),
                               **run_kwargs)
    out = np.empty((B,), dtype=np.float32)
    for c in range(N_CORES):
        out[c * NB_CORE:(c + 1) * NB_CORE] = res.results[c]["out"].T.reshape(-1)
    if run_kwargs:
        return out, res
    return out
